# revision 21
# baseline (speedup 1.0000x reference)
"""MultiHeadAttention (B=32, S=512, H=8, dk=64, dmodel=128) Trainium2 Bass kernel.

Data-parallel over batch: 8 cores x 4 batches. Returns (out, attn) like the
reference. Self-contained: hardcodes all shapes.
"""

import numpy as np

import concourse.bass as bass
import concourse.bacc as bacc
import concourse.tile as tile
from concourse import mybir
from concourse.bass_utils import run_bass_kernel_spmd
from concourse.masks import make_identity

F32 = mybir.dt.float32
F32R = mybir.dt.float32r
BF16 = mybir.dt.bfloat16
U8 = mybir.dt.uint8

B, S, DM = 32, 512, 128
H, DK = 8, 64
NCORES = 8
BL = B // NCORES  # batches per core
NT = S // 128  # 4 token tiles
EPS = 1e-5
NEGV = -29952.0  # exact in bf16; exp(0.125*(s+NEGV)) == 0.0 in f32


def _patch_act_tables():
    """Keep Exp/Ln servable only by natural_log_exp_and_others so the
    act-table-load pass picks one set for both and stops thrashing.
    Entry order (act_func_set_id) is preserved."""
    import concourse.bacc as bacc_mod
    from concourse import hw_specs as hw_mod

    if getattr(bacc_mod, "_ant_act_tables_patched", False):
        return
    orig = hw_mod.get_activation_tables
    A = mybir.ActivationFunctionType

    def patched(module_arch):
        tabs = orig(module_arch)
        out = {}
        for name, funcs in tabs.items():
            f = set(funcs)
            if name != "natural_log_exp_and_others":
                f.discard(A.Exp)
                f.discard(A.Ln)
            out[name] = f
        return out

    bacc_mod.get_activation_tables = patched
    bacc_mod._ant_act_tables_patched = True


def build(nc):
    """Build the per-core program (BL batches)."""
    _patch_act_tables()
    # ---- I/O ----
    xq_d = nc.dram_tensor("input_Q", [BL, S, DM], F32, kind="ExternalInput").ap()
    xk_d = nc.dram_tensor("input_K", [BL, S, DM], F32, kind="ExternalInput").ap()
    xv_d = nc.dram_tensor("input_V", [BL, S, DM], F32, kind="ExternalInput").ap()
    mk_d = nc.dram_tensor("attn_mask", [BL, S, S], U8, kind="ExternalInput").ap()
    wq_d = nc.dram_tensor("W_Q", [DM, H * DK], F32, kind="ExternalInput").ap()
    wk_d = nc.dram_tensor("W_K", [DM, H * DK], F32, kind="ExternalInput").ap()
    wv_d = nc.dram_tensor("W_V", [DM, H * DK], F32, kind="ExternalInput").ap()
    wfc_d = nc.dram_tensor("W_fc", [H * DK, DM], F32, kind="ExternalInput").ap()
    out_d = nc.dram_tensor("out", [BL, S, DM], F32, kind="ExternalOutput").ap()
    attn_d = nc.dram_tensor("attn", [BL, H, S, S], F32, kind="ExternalOutput").ap()

    with tile.TileContext(nc) as tc:
        _body(nc, tc, xq_d, xk_d, xv_d, mk_d, wq_d, wk_d, wv_d, wfc_d, out_d, attn_d)
    nc.compile()
    return nc


def _body(nc, tc, xq_d, xk_d, xv_d, mk_d, wq_d, wk_d, wv_d, wfc_d, out_d, attn_d):
    from contextlib import ExitStack

    ctx = ExitStack()
    const = ctx.enter_context(tc.tile_pool(name="const", bufs=1))
    wpool = ctx.enter_context(tc.tile_pool(name="w", bufs=1))
    inp = ctx.enter_context(tc.tile_pool(name="inp", bufs=3))
    xqpool = ctx.enter_context(tc.tile_pool(name="xqp", bufs=4))
    proj = ctx.enter_context(tc.tile_pool(name="proj", bufs=3))
    small = ctx.enter_context(tc.tile_pool(name="small", bufs=2))
    ppool = ctx.enter_context(tc.tile_pool(name="pq", bufs=10))
    ptpool = ctx.enter_context(tc.tile_pool(name="pt", bufs=2))
    apool = ctx.enter_context(tc.tile_pool(name="attn", bufs=6))
    opool = ctx.enter_context(tc.tile_pool(name="out", bufs=3))

    ps_st = ctx.enter_context(tc.tile_pool(name="ps_st", bufs=2, space="PSUM"))
    ps_sq = ctx.enter_context(tc.tile_pool(name="ps_sq", bufs=2, space="PSUM"))
    ps_cf = ctx.enter_context(tc.tile_pool(name="ps_cf", bufs=1, space="PSUM"))
    ps_ms = ctx.enter_context(tc.tile_pool(name="ps_ms", bufs=1, space="PSUM"))

    # ---- constants ----
    i128f = const.tile([128, 128], F32)
    make_identity(nc, i128f)
    i128b = const.tile([128, 128], BF16)
    make_identity(nc, i128b)
    negi = const.tile([128, 128], BF16)
    nc.gpsimd.memset(negi, 0.0)
    nc.gpsimd.affine_select(
        out=negi, in_=negi, compare_op=mybir.AluOpType.not_equal,
        fill=NEGV, base=0, pattern=[[-1, 128]], channel_multiplier=1,
    )
    neg8 = const.tile([2, 128], BF16)
    nc.gpsimd.memset(neg8, -8.0)
    eps_t = const.tile([128, 1], F32)
    nc.vector.memset(eps_t, EPS)

    # ---- weights (persistent) ----
    def _load_w_bf16(dram_ap, name):
        wf = wpool.tile([128, H * DK], F32, tag=f"{name}f")
        nc.scalar.dma_start(wf, dram_ap)
        wb = wpool.tile([128, H * DK], BF16, tag=f"{name}b")
        nc.vector.tensor_copy(wb, wf)
        return wb

    wq_sb = _load_w_bf16(wq_d, "wq")
    wk_sb = _load_w_bf16(wk_d, "wk")
    wv_sb = _load_w_bf16(wv_d, "wv")
    wfc_f = wpool.tile([128, NT, DM], F32)
    nc.scalar.dma_start(wfc_f, wfc_d.rearrange("(j p) d -> p j d", p=128))
    wfc_b = wpool.tile([128, NT, DM], BF16)
    nc.vector.tensor_copy(wfc_b.rearrange("p a b -> p (a b)"),
                          wfc_f.rearrange("p a b -> p (a b)"))

    def loads(b):
        st = {}
        st["xq"] = xqpool.tile([128, NT, DM], F32, tag="xq", name="xq_nat")
        nc.scalar.dma_start(st["xq"], xq_d[b].rearrange("(t p) d -> p t d", p=128))
        st["xk"] = inp.tile([128, NT, DM], F32, tag="xk", name="xk_nat")
        nc.scalar.dma_start(st["xk"], xk_d[b].rearrange("(t p) d -> p t d", p=128))
        st["xv"] = inp.tile([128, NT, DM], F32, tag="xv", name="xv_nat")
        nc.scalar.dma_start(st["xv"], xv_d[b].rearrange("(t p) d -> p t d", p=128))
        st["mk"] = inp.tile([128, NT, S], U8, tag="mk", name="mask_u8")
        nc.scalar.dma_start(st["mk"], mk_d[b].rearrange("(t p) k -> p t k", p=128))
        return st

    def prep1(st):
        xts = {}
        for name, nat in (("q", st["xq"]), ("k", st["xk"]), ("v", st["xv"])):
            xt = inp.tile([128, S], BF16, tag=f"xt{name}", name=f"xt{name}")
            for t in range(NT):
                pst = ps_ms.tile([128, 128], F32, tag="mst")
                nc.tensor.transpose(pst, nat[:, t, :], i128f)
                nc.vector.tensor_copy(xt[:, t * 128:(t + 1) * 128], pst)
            xts[name] = xt
        st["xts"] = xts
        mask_bf = inp.tile([128, NT, S], BF16, tag="mbf", name="mask_bf")
        nc.gpsimd.tensor_copy(mask_bf.rearrange("p a b -> p (a b)"),
                              st["mk"].rearrange("p a b -> p (a b)"))
        st["mbf"] = mask_bf

    def prep2(st):
        xts = st["xts"]
        qd = proj.tile([128, NT, S], BF16, tag="qd", name="qd")
        kd = proj.tile([128, NT, S], BF16, tag="kd", name="kd")
        vtm = proj.tile([128, NT, S], BF16, tag="v", name="vtm")
        for dst, w_sb, xt in ((qd, wq_sb, xts["q"]), (kd, wk_sb, xts["k"])):
            for g in range(2):
                pp = ps_st.tile([128, 2 * S], F32, tag="st")
                for i in range(2):
                    od = 2 * g + i
                    nc.tensor.matmul(
                        pp[:, i * S:(i + 1) * S],
                        lhsT=w_sb[:, od * 128:(od + 1) * 128],
                        rhs=xt, start=True, stop=True)
                nc.vector.tensor_copy(
                    dst[:, 2 * g:2 * g + 2, :].rearrange("p a b -> p (a b)"), pp)
        for g in range(2):
            pp = ps_st.tile([128, 2 * S], F32, tag="st")
            for i in range(2):
                t = 2 * g + i
                nc.tensor.matmul(
                    pp[:, i * S:(i + 1) * S],
                    lhsT=xts["v"][:, t * 128:(t + 1) * 128],
                    rhs=wv_sb, start=True, stop=True)
            nc.vector.tensor_copy(
                vtm[:, 2 * g:2 * g + 2, :].rearrange("p a b -> p (a b)"), pp)
        st["qd"], st["kd"], st["v"] = qd, kd, vtm

    def prep3(st):
        mask_bf = st["mbf"]
        maskt = inp.tile([128, NT, S], BF16, tag="mkt", name="maskt")
        for qt in range(NT):
            for kt in range(NT):
                pst = ps_ms.tile([128, 128], BF16, tag="mst")
                nc.tensor.transpose(
                    pst, mask_bf[:, qt, kt * 128:(kt + 1) * 128], i128b)
                nc.vector.tensor_copy(maskt[:, kt, qt * 128:(qt + 1) * 128], pst)
        st["mkt"] = maskt

    def d_init(st):
        st["rs"] = small.tile([128, NT, H], F32, tag="rs", name="rs_all")
        st["ln"] = small.tile([128, NT, H], F32, tag="ln", name="ln_all")
        st["lnrow"] = small.tile([2, H, S], BF16, tag="lnr", name="lnrow")

    def stage_d_qt(b, st, qt):
        qd, kd, mask_bf = st["qd"], st["kd"], st["mbf"]
        rs_all, ln_all = st["rs"], st["ln"]
        ptiles = []
        for h in range(H):
            bp = 64 * (h % 2)
            j = h // 2
            sq = ps_sq.tile([128, S], F32, tag="sq")
            nc.tensor.matmul(
                sq, lhsT=qd[bp:bp + 64, j, qt * 128:(qt + 1) * 128],
                rhs=kd[bp:bp + 64, j, :], start=True, stop=False)
            nc.tensor.matmul(sq, lhsT=negi, rhs=mask_bf[:, qt, :],
                             start=False, stop=True)
            p_sb = ppool.tile([128, S], F32, tag="p")
            nc.scalar.activation(
                out=p_sb, in_=sq, func=mybir.ActivationFunctionType.Exp,
                scale=0.125, accum_out=rs_all[:, qt, h:h + 1])
            ptiles.append(p_sb)
        r_qt = small.tile([128, H], F32, tag="rq")
        nc.vector.reciprocal(r_qt, rs_all[:, qt, :])
        nc.scalar.activation(out=ln_all[:, qt, :], in_=rs_all[:, qt, :],
                             func=mybir.ActivationFunctionType.Ln)
        lnq = small.tile([128, 16], BF16, tag="lnq")
        nc.vector.tensor_copy(lnq[:, 0:8], ln_all[:, qt, :])
        hi_f = small.tile([128, 8], F32, tag="hif")
        nc.vector.tensor_copy(hi_f, lnq[:, 0:8])
        lo_f = small.tile([128, 8], F32, tag="lof")
        nc.vector.tensor_sub(lo_f, ln_all[:, qt, :], hi_f)
        nc.vector.tensor_copy(lnq[:, 8:16], lo_f)
        pst = ps_ms.tile([16, 128], BF16, tag="mst")
        nc.tensor.transpose(pst, lnq, i128b)
        ltq = small.tile([16, 128], BF16, tag="ltq")
        nc.vector.tensor_copy(ltq, pst)
        lnrow = st["lnrow"]
        nc.scalar.dma_start(lnrow[0:1, :, qt * 128:(qt + 1) * 128], ltq[0:8, :])
        nc.scalar.dma_start(lnrow[1:2, :, qt * 128:(qt + 1) * 128], ltq[8:16, :])
        for h in range(H):
            at = apool.tile([128, S], F32, tag="at")
            nc.vector.tensor_scalar_mul(at, ptiles[h], r_qt[:, h:h + 1])
            eng = nc.sync if h % 2 == 0 else nc.gpsimd
            eng.dma_start(attn_d[b, h, qt * 128:(qt + 1) * 128, :], at)

    def stage_c_heads(st, heads):
        qd, kd, vtm, maskt, lnrow = (st["qd"], st["kd"], st["v"], st["mkt"],
                                     st["lnrow"])
        if "ct" not in st:
            st["ct"] = proj.tile([128, NT, S], BF16, tag="ct", name="ct")
        ct = st["ct"]
        for h in heads:
            bp = 64 * (h % 2)
            j = h // 2
            pt = ptpool.tile([128, NT, S], BF16, tag="pt")
            for g in range(2):
                stp = ps_st.tile([128, 2 * S], F32, tag="st")
                for i in range(2):
                    kt = 2 * g + i
                    sl = stp[:, i * S:(i + 1) * S]
                    nc.tensor.matmul(
                        sl, lhsT=kd[bp:bp + 64, j, kt * 128:(kt + 1) * 128],
                        rhs=qd[bp:bp + 64, j, :], start=True, stop=False)
                    nc.tensor.matmul(sl, lhsT=neg8, rhs=lnrow[:, h, :],
                                     start=False, stop=False)
                    nc.tensor.matmul(sl, lhsT=negi, rhs=maskt[:, kt, :],
                                     start=False, stop=True)
                ptg = pt[:, 2 * g:2 * g + 2, :].rearrange("p a b -> p (a b)")
                nc.scalar.activation(out=ptg, in_=stp,
                                     func=mybir.ActivationFunctionType.Exp,
                                     scale=0.125)
            cx = ps_cf.tile([64, S], F32, tag="cf")
            for kt in range(NT):
                nc.tensor.matmul(
                    cx, lhsT=vtm[:, kt, h * DK:(h + 1) * DK], rhs=pt[:, kt, :],
                    start=(kt == 0), stop=(kt == NT - 1))
            nc.vector.tensor_copy(ct[bp:bp + 64, j, :], cx)

    def stage_e_qt(b, st, qt):
        ct, xq_nat = st["ct"], st["xq"]
        fc = ps_cf.tile([128, DM], F32, tag="cf")
        for j in range(NT):
            nc.tensor.matmul(fc, lhsT=ct[:, j, qt * 128:(qt + 1) * 128],
                             rhs=wfc_b[:, j, :], start=(j == 0),
                             stop=(j == NT - 1))
        res = opool.tile([128, DM], F32, tag="res")
        nc.vector.tensor_add(res, fc, xq_nat[:, qt, :])
        stats = small.tile([128, 6], F32, tag="bns")
        nc.vector.bn_stats(stats, res)
        mv = small.tile([128, 2], F32, tag="bna")
        nc.vector.bn_aggr(mv, stats)
        lnv = small.tile([128, 1], F32, tag="lnv")
        nc.scalar.activation(out=lnv, in_=mv[:, 1:2],
                             func=mybir.ActivationFunctionType.Ln, bias=eps_t)
        rstd = small.tile([128, 1], F32, tag="rsd")
        nc.scalar.activation(out=rstd, in_=lnv,
                             func=mybir.ActivationFunctionType.Exp, scale=-0.5)
        o_sb = opool.tile([128, DM], F32, tag="o")
        nc.vector.tensor_scalar(
            out=o_sb, in0=res, scalar1=mv[:, 0:1], scalar2=rstd,
            op0=mybir.AluOpType.subtract, op1=mybir.AluOpType.mult)
        nc.scalar.dma_start(out_d[b, qt * 128:(qt + 1) * 128, :], o_sb)

    # Software pipeline: zipper D(b+1) | C(b) | E(b-1) | setup-piece(b+2)
    states = {}

    def piece(b, i):
        if i == 0:
            states[b] = loads(b)
        elif i == 1:
            prep1(states[b])
        elif i == 2:
            prep2(states[b])
        else:
            prep3(states[b])
            d_init(states[b])

    states[0] = loads(0)
    prep1(states[0])
    prep2(states[0])
    prep3(states[0])
    d_init(states[0])
    for qt in range(NT):
        stage_d_qt(0, states[0], qt)
        if BL > 1:
            piece(1, qt)
    for b in range(BL):
        for i in range(NT):
            if b + 1 < BL:
                stage_d_qt(b + 1, states[b + 1], i)
            stage_c_heads(states[b], [2 * i, 2 * i + 1])
            if b > 0:
                stage_e_qt(b - 1, states[b - 1], i)
            if b + 2 < BL:
                piece(b + 2, i)
        if b > 0:
            del states[b - 1]
    for i in range(NT):
        stage_e_qt(BL - 1, states[BL - 1], i)

    ctx.close()


_NC_CACHE = None


def _get_nc():
    global _NC_CACHE
    if _NC_CACHE is None:
        nc = bacc.Bacc("TRN2", target_bir_lowering=False, debug=False,
                       num_devices=NCORES)
        _NC_CACHE = build(nc)
    return _NC_CACHE


def kernel(**inputs):
    nc = _get_nc()
    mask = np.ascontiguousarray(inputs["attn_mask"]).view(np.uint8)
    in_maps = []
    for c in range(NCORES):
        sl = slice(c * BL, (c + 1) * BL)
        in_maps.append({
            "input_Q": np.ascontiguousarray(inputs["input_Q"][sl]),
            "input_K": np.ascontiguousarray(inputs["input_K"][sl]),
            "input_V": np.ascontiguousarray(inputs["input_V"][sl]),
            "attn_mask": np.ascontiguousarray(mask[sl]),
            "W_Q": np.ascontiguousarray(inputs["W_Q"]),
            "W_K": np.ascontiguousarray(inputs["W_K"]),
            "W_V": np.ascontiguousarray(inputs["W_V"]),
            "W_fc": np.ascontiguousarray(inputs["W_fc"]),
        })
    res = run_bass_kernel_spmd(nc, in_maps, core_ids=list(range(NCORES)))
    out = np.concatenate([r["out"] for r in res.results], axis=0)
    attn = np.concatenate([r["attn"] for r in res.results], axis=0)
    return out, attn
